# revision 14
# baseline (speedup 1.0000x reference)
"""GCN (2-layer + linear residual) Trainium2 kernel, 8 NeuronCores.

Strategy (graph/data parallel, per the sharding hint):
  - Nodes are partitioned contiguously across 8 cores (12500 each).
  - Per layer l: each core computes its slice of u_l = g_l @ W_l (bf16),
    AllGathers the slices into a full per-core DRAM table [N,128] bf16,
    then aggregates its own nodes' incoming edges: batched dma_gather of
    u_l[src] rows and a one-hot "selection" matmul that collapses edges
    onto dst nodes, with the GCN norm folded into the selection matrix:
    S[e,p] = norm_e * (r_e == p), built in ONE vector op
    tensor_scalar(iota, r_e, nu_e, is_equal, mult).
  - Aggregation is tiled over static 128-node windows (window w covers
    dst nodes [128w, 128w+128)). dma_gather uses int16 indices, so the
    table is split into <=32768-row regions; per (window, region) there
    are C_q subtiles of 128 edge slots (C_q = global max, so the SPMD
    program is identical on all cores; unused slots are padded with -1
    indices, which the gather skips at zero DMA cost and whose nu=0
    entries zero them out of the selection matmul). All of one window's
    subtile matmuls accumulate in a single PSUM tile; the epilogue
    relu(psum + b) lands feature-major in SBUF staging that feeds the
    next layer's matmul directly.
  - Residual x @ Wfc + bfc overlaps with the collectives; the final
    output is written feature-major [128, 12500] per core and transposed
    on the host.
"""

import os
import sys

import numpy as np

if "/opt/trn_rl_repo" not in sys.path:
    sys.path.insert(0, "/opt/trn_rl_repo")

import ml_dtypes

BF16 = ml_dtypes.bfloat16

P = 128          # partitions / feature dim / window width
D = 128          # feature dim
NCORES = 8
REG_ROWS = 32768   # dma_gather int16 index reach per table region
B = 8            # windows per gather batch
WARM_BATCHES = 8   # early batches pad with index 0 (not -1) to init SBUF

_LAST_RESULTS = {}   # test introspection: exec_time etc.


def _regions(N):
    regs = []
    q0 = 0
    while q0 < N:
        regs.append((q0, min(q0 + REG_ROWS, N)))
        q0 += REG_ROWS
    return regs


def _batches(nwin):
    out = []
    w0 = 0
    while w0 < nwin:
        out.append((w0, min(B, nwin - w0)))
        w0 += B
    return out


# --------------------------------------------------------------------------
# Host-side preprocessing
# --------------------------------------------------------------------------

def _core_edges(core, src, dst, nu, npc):
    lo = core * npc
    sel = (dst >= lo) & (dst < lo + npc)
    s_src = src[sel]
    s_dst = dst[sel] - lo
    s_nu = nu[sel]
    order = np.argsort(s_dst, kind="stable")
    return s_src[order], s_dst[order], s_nu[order]


def _count_wr(s_src, s_dst, npc, N):
    """Edge counts per (window, region)."""
    nwin = (npc + P - 1) // P
    nreg = len(_regions(N))
    w = s_dst // P
    q = s_src // REG_ROWS
    cnt = np.zeros((nwin, nreg), np.int64)
    np.add.at(cnt, (w, q), 1)
    return cnt


def _finalize_core(s_src, s_dst, s_nu, npc, N, C):
    """Build per-core device arrays for region caps C (list per region).

    Returns idx [P, S*8] int16 (wrapped+replicated, op-major layout),
    r [P, S] f32, nu [P, S] f32, counts [1, n_ops] i32.
    """
    nwin = (npc + P - 1) // P
    regs = _regions(N)
    nreg = len(regs)
    SUBW = sum(C)
    offq = np.concatenate([[0], np.cumsum(C)])
    S = nwin * SUBW

    slot_r = np.zeros(S * P, dtype=np.float32)
    slot_nu = np.zeros(S * P, dtype=np.float32)
    flat_idx = np.full(S * P, -1, dtype=np.int16)

    w_of = s_dst // P
    q_of = s_src // REG_ROWS
    batches = _batches(nwin)

    counts_ops = []
    for b, (w0, bw) in enumerate(batches):
        pad_val = 0
        for q in range(nreg):
            if C[q] == 0:
                counts_ops.append(0)
                continue
            sub0 = w0 * SUBW + bw * offq[q]
            n_in_op = 0
            for wl in range(bw):
                w = w0 + wl
                m = (w_of == w) & (q_of == q)
                e_src = s_src[m]
                e_dst = s_dst[m]
                e_nu = s_nu[m]
                ne = len(e_src)
                assert ne <= C[q] * P
                sub_base = sub0 + wl * C[q]
                o = sub_base * P
                flat_idx[o:o + C[q] * P] = pad_val
                flat_idx[o:o + ne] = (e_src - regs[q][0]).astype(np.int16)
                slot_r[o:o + ne] = (e_dst - w * P).astype(np.float32)
                slot_nu[o:o + ne] = e_nu
                n_in_op += ne if pad_val < 0 else C[q] * P
            counts_ops.append(n_in_op)
    r = np.ascontiguousarray(slot_r.reshape(S, P).T)
    nnu = np.ascontiguousarray(slot_nu.reshape(S, P).T)

    # wrapped indices: the flat slot order IS the op order (op-major
    # layout); wrap each op's span into 16 partitions, replicate to 128.
    idx16 = np.zeros((16, S * 8), np.int16)
    for b, (w0, bw) in enumerate(batches):
        for q in range(nreg):
            if C[q] == 0:
                continue
            sub0 = w0 * SUBW + bw * offq[q]
            ln = bw * C[q] * P
            span = flat_idx[sub0 * P: sub0 * P + ln]
            idx16[:, sub0 * 8: sub0 * 8 + ln // 16] = (
                span.reshape(ln // 16, 16).T
            )
    idx = np.ascontiguousarray(np.tile(idx16, (8, 1)))
    counts = np.asarray(counts_ops, np.int32).reshape(1, -1)
    return idx, r, nnu, counts


# --------------------------------------------------------------------------
# Device program
# --------------------------------------------------------------------------

def _build_program(N, npc, C):
    from contextlib import ExitStack

    import concourse.bass as bass
    import concourse.tile as tile
    from concourse import bacc, mybir
    from concourse.tile_rust import add_dep_helper

    f32 = mybir.dt.float32
    bf16 = mybir.dt.bfloat16
    i32 = mybir.dt.int32
    i16 = mybir.dt.int16
    AF = mybir.ActivationFunctionType
    ALU = mybir.AluOpType

    nwin = (npc + P - 1) // P
    regs = _regions(N)
    nreg = len(regs)
    SUBW = sum(C)
    offq = [0]
    for q in range(nreg):
        offq.append(offq[-1] + C[q])
    S = nwin * SUBW
    batches = _batches(nwin)
    n_ops = len(batches) * nreg
    npc_pad = nwin * P
    RW = 512
    nrchunks = (npc + RW - 1) // RW

    nc = bacc.Bacc(
        "TRN2",
        target_bir_lowering=False,
        debug=False,
        num_devices=NCORES,
    )

    # ---- I/O ----
    xT_e = nc.dram_tensor("xT", [P, npc], bf16, kind="ExternalInput")
    W1_e = nc.dram_tensor("W1", [D, D], bf16, kind="ExternalInput")
    W2_e = nc.dram_tensor("W2", [D, D], bf16, kind="ExternalInput")
    Wfc_e = nc.dram_tensor("Wfc", [D, D], bf16, kind="ExternalInput")
    b1_e = nc.dram_tensor("b1", [P, 1], f32, kind="ExternalInput")
    b2_e = nc.dram_tensor("b2", [P, 1], f32, kind="ExternalInput")
    bfc_e = nc.dram_tensor("bfc", [P, 1], f32, kind="ExternalInput")
    iota_e = nc.dram_tensor("iota", [P, P], bf16, kind="ExternalInput")
    idx_e = nc.dram_tensor("idx", [P, S * 8], i16, kind="ExternalInput")
    r_e = nc.dram_tensor("r", [P, S], f32, kind="ExternalInput")
    nu_e = nc.dram_tensor("nu", [P, S], f32, kind="ExternalInput")
    cnt_e = nc.dram_tensor("cnt", [1, n_ops], i32, kind="ExternalInput")
    out_e = nc.dram_tensor("out", [P, npc], f32, kind="ExternalOutput")

    # ---- internal DRAM ----
    t1 = nc.dram_tensor("table1", [N, D], bf16, addr_space="Shared")
    t2 = nc.dram_tensor("table2", [N, D], bf16, addr_space="Shared")
    bnc1 = nc.dram_tensor("bounce1", [npc_pad, D], bf16)
    bnc2 = nc.dram_tensor("bounce2", [npc_pad, D], bf16)

    rgroups = [list(range(NCORES))]

    with tile.TileContext(nc) as tc, ExitStack() as ctx:
        cpool = ctx.enter_context(tc.tile_pool(name="const", bufs=1))
        gpool = ctx.enter_context(tc.tile_pool(name="gather", bufs=2))
        spool = ctx.enter_context(tc.tile_pool(name="sel", bufs=8))
        stpool = ctx.enter_context(tc.tile_pool(name="stage", bufs=1))
        ckpool = ctx.enter_context(tc.tile_pool(name="chunk", bufs=4))
        pspool = ctx.enter_context(tc.tile_pool(name="ps", bufs=2, space="PSUM"))
        apspool = ctx.enter_context(tc.tile_pool(name="aps", bufs=3, space="PSUM"))
        rpspool = ctx.enter_context(tc.tile_pool(name="rps", bufs=2, space="PSUM"))

        def load_const(ext, shape, dtype):
            t = cpool.tile(shape, dtype, tag=ext.name + "_sb")
            nc.sync.dma_start(out=t[:], in_=ext[:, :])
            return t

        xT = load_const(xT_e, [P, npc], bf16)
        W1 = load_const(W1_e, [D, D], bf16)
        W2 = load_const(W2_e, [D, D], bf16)
        Wfc = load_const(Wfc_e, [D, D], bf16)
        b1 = load_const(b1_e, [P, 1], f32)
        b2 = load_const(b2_e, [P, 1], f32)
        bfc = load_const(bfc_e, [P, 1], f32)
        iot = load_const(iota_e, [P, P], bf16)
        idxm = load_const(idx_e, [P, S * 8], i16)
        rm = load_const(r_e, [P, S], f32)
        num = load_const(nu_e, [P, S], f32)
        cntm = load_const(cnt_e, [1, n_ops], i32)

        stag1 = stpool.tile([P, npc], bf16, tag="stag1")
        stag2 = stpool.tile([P, npc], f32, tag="stag2")

        def production(g_sbuf, W_sb, bounce):
            """u = (g @ W) per 128-node chunk -> bf16 -> bounce DRAM."""
            for c in range(nwin):
                c0 = c * P
                cn = min(P, npc - c0)
                ps = pspool.tile([P, P], f32, space="PSUM", tag="pps")
                nc.tensor.matmul(
                    out=ps[:cn, :],
                    lhsT=g_sbuf[:, c0:c0 + cn],
                    rhs=W_sb[:],
                    start=True,
                    stop=True,
                )
                ck = ckpool.tile([P, P], bf16, tag="prodck")
                nc.scalar.activation(ck[:cn, :], ps[:cn, :], AF.Copy)
                nc.sync.dma_start(out=bounce[c0:c0 + cn, :], in_=ck[:cn, :])

        def aggregate(table, bias_sb, stag):
            """Gather + selection-matmul + relu epilogue into stag."""
            for b, (w0, bw) in enumerate(batches):
                gtiles = []
                for q in range(nreg):
                    if C[q] == 0:
                        gtiles.append(None)
                        continue
                    sub0 = w0 * SUBW + bw * offq[q]
                    ln = bw * C[q] * P
                    gb = gpool.tile([P, B * C[q] * D], bf16, tag=f"gbuf{q}")
                    nc.gpsimd.dma_gather(
                        gb[:, :ln // P * D].rearrange("p (c d) -> p c d", d=D),
                        table[regs[q][0]:regs[q][1], :],
                        idxm[:, sub0 * 8: sub0 * 8 + ln // 16],
                        ln,
                        ln,
                        D,
                        single_packet=False,
                    )
                    gtiles.append((gb, sub0))
                for wl in range(bw):
                    w = w0 + wl
                    n0 = w * P
                    wn = min(P, npc - n0)
                    ps = apspool.tile([P, P], f32, space="PSUM", tag="aps")
                    k = 0
                    for q in range(nreg):
                        if C[q] == 0:
                            continue
                        gb, sub0 = gtiles[q]
                        for s in range(C[q]):
                            sub = sub0 + wl * C[q] + s
                            lsub = wl * C[q] + s
                            Sp = spool.tile([P, P], bf16, tag="selm")
                            nc.vector.tensor_scalar(
                                Sp[:],
                                iot[:],
                                rm[:, sub:sub + 1],
                                num[:, sub:sub + 1],
                                ALU.is_equal,
                                ALU.mult,
                            )
                            nc.tensor.matmul(
                                out=ps[:],
                                lhsT=gb[:, lsub * D:(lsub + 1) * D],
                                rhs=Sp[:],
                                start=(k == 0),
                                stop=(k == SUBW - 1),
                            )
                            k += 1
                    nc.scalar.activation(
                        stag[:, n0:n0 + wn],
                        ps[:, :wn],
                        AF.Relu,
                        bias=bias_sb[:, 0:1],
                    )

        # ---------------- layer 1 ----------------
        production(xT, W1, bnc1)
        nc.gpsimd.collective_compute(
            "AllGather",
            ALU.bypass,
            replica_groups=rgroups,
            ins=[bnc1[0:npc, :].opt()],
            outs=[t1[0:NCORES * npc, :].opt()],
        )
        aggregate(t1, b1, stag1)

        # ---------------- layer 2 ----------------
        production(stag1, W2, bnc2)
        nc.gpsimd.collective_compute(
            "AllGather",
            ALU.bypass,
            replica_groups=rgroups,
            ins=[bnc2[0:npc, :].opt()],
            outs=[t2[0:NCORES * npc, :].opt()],
        )
        aggregate(t2, b2, stag2)

        # ---------------- residual + combine ----------------
        for rc in range(nrchunks):
            r0 = rc * RW
            cw = min(RW, npc - r0)
            ps = rpspool.tile([P, RW], f32, space="PSUM", tag="rps")
            nc.tensor.matmul(
                out=ps[:, :cw],
                lhsT=Wfc[:],
                rhs=xT[:, r0:r0 + cw],
                start=True,
                stop=True,
            )
            rb = ckpool.tile([P, RW], f32, tag="resck")
            nc.scalar.activation(
                rb[:, :cw], ps[:, :cw], AF.Identity, bias=bfc[:, 0:1]
            )
            ob = ckpool.tile([P, RW], f32, tag="outck")
            nc.vector.tensor_tensor(
                out=ob[:, :cw],
                in0=rb[:, :cw],
                in1=stag2[:, r0:r0 + cw],
                op=ALU.add,
            )
            nc.sync.dma_start(out=out_e[:, r0:r0 + cw], in_=ob[:, :cw])

    nc.compile()
    return nc


# --------------------------------------------------------------------------
# Entry point
# --------------------------------------------------------------------------

def _prep(x, edge_index, W1, b1, W2, b2, Wfc, bfc):
    N = x.shape[0]
    assert N % NCORES == 0
    npc = N // NCORES

    loop = np.arange(N, dtype=np.int64)
    src = np.concatenate([edge_index[0].astype(np.int64), loop])
    dst = np.concatenate([edge_index[1].astype(np.int64), loop])
    deg = np.bincount(dst, minlength=N).astype(np.float32)
    sigma = np.where(deg > 0, 1.0 / np.sqrt(deg), 0.0).astype(np.float32)
    nu = sigma[src] * sigma[dst]

    cores = [_core_edges(c, src, dst, nu, npc) for c in range(NCORES)]

    nreg = len(_regions(N))
    maxc = np.zeros(nreg, np.int64)
    for s_src, s_dst, s_nu in cores:
        cnt = _count_wr(s_src, s_dst, npc, N)
        maxc = np.maximum(maxc, cnt.max(axis=0))
    C = [int((m + P - 1) // P) for m in maxc]

    iota = np.tile(np.arange(P, dtype=np.float32), (P, 1)).astype(BF16)
    W1b = np.asarray(W1, np.float32).astype(BF16)
    W2b = np.asarray(W2, np.float32).astype(BF16)
    Wfcb = np.asarray(Wfc, np.float32).astype(BF16)
    b1c = np.asarray(b1, np.float32).reshape(P, 1)
    b2c = np.asarray(b2, np.float32).reshape(P, 1)
    bfcc = np.asarray(bfc, np.float32).reshape(P, 1)

    in_maps = []
    for c in range(NCORES):
        s_src, s_dst, s_nu = cores[c]
        idx, r, nnu, counts = _finalize_core(s_src, s_dst, s_nu, npc, N, C)
        xTc = np.ascontiguousarray(x[c * npc:(c + 1) * npc].T.astype(BF16))
        in_maps.append({
            "xT": xTc,
            "W1": W1b, "W2": W2b, "Wfc": Wfcb,
            "b1": b1c, "b2": b2c, "bfc": bfcc,
            "iota": iota,
            "idx": idx, "r": r, "nu": nnu, "cnt": counts,
        })
    return in_maps, N, npc, C


def _ensure_ntff_hook():
    """The agent image's antenv lacks axon_hooks; shim it so trace=True
    works (falls back to hookless if the profiling lib is unavailable)."""
    try:
        import antenv.axon_hooks  # noqa: F401
        return
    except ImportError:
        pass
    try:
        import types

        import antenv

        mod = types.ModuleType("antenv.axon_hooks")
        _hook = [None]
        mod.set_axon_ntff_profile_hook = lambda h: _hook.__setitem__(0, h)
        mod.get_axon_ntff_profile_hook = lambda: _hook[0]
        sys.modules["antenv.axon_hooks"] = mod
        antenv.axon_hooks = mod
        try:
            from trn_agent_boot.trn_boot import _ntff_profile_via_ctypes

            mod.set_axon_ntff_profile_hook(
                _ntff_profile_via_ctypes("/opt/axon/libaxon_pjrt.so")
            )
        except Exception:
            pass
    except Exception:
        pass


def kernel(x, edge_index, W1, b1, W2, b2, Wfc, bfc):
    from concourse.bass_utils import run_bass_kernel_spmd

    x = np.asarray(x, np.float32)
    edge_index = np.asarray(edge_index)
    in_maps, N, npc, C = _prep(x, edge_index, W1, b1, W2, b2, Wfc, bfc)
    nc = _build_program(N, npc, C)

    trace = os.environ.get("GNN_TRACE", "0") == "1"
    if trace:
        _ensure_ntff_hook()
    res = run_bass_kernel_spmd(
        nc, in_maps, core_ids=list(range(NCORES)), trace=trace
    )
    _LAST_RESULTS["exec_time_ns"] = res.exec_time_ns
    _LAST_RESULTS["mean_exec_time_ns"] = res.mean_exec_time_ns
    _LAST_RESULTS["trace"] = res.instructions_and_trace

    out = np.concatenate(
        [res.results[c]["out"].T for c in range(NCORES)], axis=0
    )
    return np.ascontiguousarray(out.astype(np.float32))


# revision 15
# speedup vs baseline: 1.3280x; 1.3280x over previous
"""GCN (2-layer + linear residual) Trainium2 kernel, 8 NeuronCores.

Strategy (graph/data parallel, per the sharding hint):
  - Nodes are partitioned contiguously across 8 cores (12500 each).
  - Per layer l: each core computes its slice of u_l = g_l @ W_l (bf16),
    AllGathers the slices into a full per-core DRAM table [N,128] bf16,
    then aggregates its own nodes' incoming edges: batched dma_gather of
    u_l[src] rows and a one-hot "selection" matmul that collapses edges
    onto dst nodes, with the GCN norm folded into the selection matrix:
    S[e,p] = norm_e * (r_e == p), built in ONE vector op
    tensor_scalar(iota, r_e, nu_e, is_equal, mult).
  - Aggregation is tiled over static 128-node windows (window w covers
    dst nodes [128w, 128w+128)). dma_gather uses int16 indices, so the
    table is split into <=32768-row regions; per (window, region) there
    are C_q subtiles of 128 edge slots (C_q = global max, so the SPMD
    program is identical on all cores; unused slots are padded with -1
    indices, which the gather skips at zero DMA cost and whose nu=0
    entries zero them out of the selection matmul). All of one window's
    subtile matmuls accumulate in a single PSUM tile; the epilogue
    relu(psum + b) lands feature-major in SBUF staging that feeds the
    next layer's matmul directly.
  - Residual x @ Wfc + bfc overlaps with the collectives; the final
    output is written feature-major [128, 12500] per core and transposed
    on the host.
"""

import os
import sys

import numpy as np

if "/opt/trn_rl_repo" not in sys.path:
    sys.path.insert(0, "/opt/trn_rl_repo")

import ml_dtypes

BF16 = ml_dtypes.bfloat16

P = 128          # partitions / feature dim
D = 128          # feature dim
WW = 256         # aggregation window width (dst nodes per psum tile)
NCORES = 8
REG_ROWS = 32768   # dma_gather int16 index reach per table region
B = 4            # windows per gather batch
WARM_BATCHES = 8   # early batches pad with index 0 (not -1) to init SBUF

_LAST_RESULTS = {}   # test introspection: exec_time etc.


def _regions(N):
    regs = []
    q0 = 0
    while q0 < N:
        regs.append((q0, min(q0 + REG_ROWS, N)))
        q0 += REG_ROWS
    return regs


def _batches(nwin):
    out = []
    w0 = 0
    while w0 < nwin:
        out.append((w0, min(B, nwin - w0)))
        w0 += B
    return out


# --------------------------------------------------------------------------
# Host-side preprocessing
# --------------------------------------------------------------------------

def _core_edges(core, src, dst, nu, npc):
    lo = core * npc
    sel = (dst >= lo) & (dst < lo + npc)
    s_src = src[sel]
    s_dst = dst[sel] - lo
    s_nu = nu[sel]
    order = np.argsort(s_dst, kind="stable")
    return s_src[order], s_dst[order], s_nu[order]


def _count_wr(s_src, s_dst, npc, N):
    """Edge counts per (window, region)."""
    nwin = (npc + WW - 1) // WW
    nreg = len(_regions(N))
    w = s_dst // WW
    q = s_src // REG_ROWS
    cnt = np.zeros((nwin, nreg), np.int64)
    np.add.at(cnt, (w, q), 1)
    return cnt


def _finalize_core(s_src, s_dst, s_nu, npc, N, C):
    """Build per-core device arrays for region caps C (list per region).

    Returns idx [P, S*8] int16 (wrapped+replicated, op-major layout),
    r [P, S] f32, nu [P, S] f32, counts [1, n_ops] i32.
    """
    nwin = (npc + WW - 1) // WW
    regs = _regions(N)
    nreg = len(regs)
    SUBW = sum(C)
    offq = np.concatenate([[0], np.cumsum(C)])
    S = nwin * SUBW

    slot_r = np.zeros(S * P, dtype=np.float32)
    slot_nu = np.zeros(S * P, dtype=np.float32)
    flat_idx = np.full(S * P, -1, dtype=np.int16)

    w_of = s_dst // WW
    q_of = s_src // REG_ROWS
    batches = _batches(nwin)

    counts_ops = []
    for b, (w0, bw) in enumerate(batches):
        pad_val = 0
        for q in range(nreg):
            if C[q] == 0:
                counts_ops.append(0)
                continue
            sub0 = w0 * SUBW + bw * offq[q]
            n_in_op = 0
            for wl in range(bw):
                w = w0 + wl
                m = (w_of == w) & (q_of == q)
                e_src = s_src[m]
                e_dst = s_dst[m]
                e_nu = s_nu[m]
                ne = len(e_src)
                assert ne <= C[q] * P
                sub_base = sub0 + wl * C[q]
                o = sub_base * P
                flat_idx[o:o + C[q] * P] = pad_val
                flat_idx[o:o + ne] = (e_src - regs[q][0]).astype(np.int16)
                slot_r[o:o + ne] = (e_dst - w * WW).astype(np.float32)
                slot_nu[o:o + ne] = e_nu
                n_in_op += ne if pad_val < 0 else C[q] * P
            counts_ops.append(n_in_op)
    r = np.ascontiguousarray(slot_r.reshape(S, P).T)
    nnu = np.ascontiguousarray(slot_nu.reshape(S, P).T)

    # wrapped indices: the flat slot order IS the op order (op-major
    # layout); wrap each op's span into 16 partitions, replicate to 128.
    idx16 = np.zeros((16, S * 8), np.int16)
    for b, (w0, bw) in enumerate(batches):
        for q in range(nreg):
            if C[q] == 0:
                continue
            sub0 = w0 * SUBW + bw * offq[q]
            ln = bw * C[q] * P
            span = flat_idx[sub0 * P: sub0 * P + ln]
            idx16[:, sub0 * 8: sub0 * 8 + ln // 16] = (
                span.reshape(ln // 16, 16).T
            )
    idx = np.ascontiguousarray(np.tile(idx16, (8, 1)))
    counts = np.asarray(counts_ops, np.int32).reshape(1, -1)
    return idx, r, nnu, counts


# --------------------------------------------------------------------------
# Device program
# --------------------------------------------------------------------------

def _build_program(N, npc, C):
    from contextlib import ExitStack

    import concourse.bass as bass
    import concourse.tile as tile
    from concourse import bacc, mybir
    from concourse.tile_rust import add_dep_helper

    f32 = mybir.dt.float32
    bf16 = mybir.dt.bfloat16
    i32 = mybir.dt.int32
    i16 = mybir.dt.int16
    AF = mybir.ActivationFunctionType
    ALU = mybir.AluOpType

    nwin = (npc + WW - 1) // WW
    nchunks = (npc + P - 1) // P
    regs = _regions(N)
    nreg = len(regs)
    SUBW = sum(C)
    offq = [0]
    for q in range(nreg):
        offq.append(offq[-1] + C[q])
    S = nwin * SUBW
    batches = _batches(nwin)
    n_ops = len(batches) * nreg
    npc_pad = nchunks * P
    RW = 512
    nrchunks = (npc + RW - 1) // RW

    nc = bacc.Bacc(
        "TRN2",
        target_bir_lowering=False,
        debug=False,
        num_devices=NCORES,
    )

    # ---- I/O ----
    xT_e = nc.dram_tensor("xT", [P, npc], bf16, kind="ExternalInput")
    W1_e = nc.dram_tensor("W1", [D, D], bf16, kind="ExternalInput")
    W2_e = nc.dram_tensor("W2", [D, D], bf16, kind="ExternalInput")
    Wfc_e = nc.dram_tensor("Wfc", [D, D], bf16, kind="ExternalInput")
    b1_e = nc.dram_tensor("b1", [P, 1], f32, kind="ExternalInput")
    b2_e = nc.dram_tensor("b2", [P, 1], f32, kind="ExternalInput")
    bfc_e = nc.dram_tensor("bfc", [P, 1], f32, kind="ExternalInput")
    iota_e = nc.dram_tensor("iota", [P, WW], f32, kind="ExternalInput")
    idx_e = nc.dram_tensor("idx", [P, S * 8], i16, kind="ExternalInput")
    r_e = nc.dram_tensor("r", [P, S], f32, kind="ExternalInput")
    nu_e = nc.dram_tensor("nu", [P, S], f32, kind="ExternalInput")
    cnt_e = nc.dram_tensor("cnt", [1, n_ops], i32, kind="ExternalInput")
    out_e = nc.dram_tensor("out", [P, npc], f32, kind="ExternalOutput")

    # ---- internal DRAM ----
    t1 = nc.dram_tensor("table1", [N, D], bf16, addr_space="Shared")
    t2 = nc.dram_tensor("table2", [N, D], bf16, addr_space="Shared")
    bnc1 = nc.dram_tensor("bounce1", [npc_pad, D], bf16)
    bnc2 = nc.dram_tensor("bounce2", [npc_pad, D], bf16)

    rgroups = [list(range(NCORES))]

    with tile.TileContext(nc) as tc, ExitStack() as ctx:
        cpool = ctx.enter_context(tc.tile_pool(name="const", bufs=1))
        gpool = ctx.enter_context(tc.tile_pool(name="gather", bufs=2))
        spool = ctx.enter_context(tc.tile_pool(name="sel", bufs=8))
        stpool = ctx.enter_context(tc.tile_pool(name="stage", bufs=1))
        ckpool = ctx.enter_context(tc.tile_pool(name="chunk", bufs=4))
        pspool = ctx.enter_context(tc.tile_pool(name="ps", bufs=2, space="PSUM"))
        apspool = ctx.enter_context(tc.tile_pool(name="aps", bufs=3, space="PSUM"))
        rpspool = ctx.enter_context(tc.tile_pool(name="rps", bufs=2, space="PSUM"))

        def load_const(ext, shape, dtype):
            t = cpool.tile(shape, dtype, tag=ext.name + "_sb")
            nc.sync.dma_start(out=t[:], in_=ext[:, :])
            return t

        xT = load_const(xT_e, [P, npc], bf16)
        W1 = load_const(W1_e, [D, D], bf16)
        W2 = load_const(W2_e, [D, D], bf16)
        Wfc = load_const(Wfc_e, [D, D], bf16)
        b1 = load_const(b1_e, [P, 1], f32)
        b2 = load_const(b2_e, [P, 1], f32)
        bfc = load_const(bfc_e, [P, 1], f32)
        iot = load_const(iota_e, [P, WW], f32)
        idxm = load_const(idx_e, [P, S * 8], i16)
        rm = load_const(r_e, [P, S], f32)
        num = load_const(nu_e, [P, S], f32)
        cntm = load_const(cnt_e, [1, n_ops], i32)

        stag1 = stpool.tile([P, npc], bf16, tag="stag1")
        stag2 = stpool.tile([P, npc], f32, tag="stag2")

        def production(g_sbuf, W_sb, bounce):
            """u = (g @ W) per 128-node chunk -> bf16 -> bounce DRAM."""
            for c in range(nchunks):
                c0 = c * P
                cn = min(P, npc - c0)
                ps = pspool.tile([P, P], f32, space="PSUM", tag="pps")
                nc.tensor.matmul(
                    out=ps[:cn, :],
                    lhsT=g_sbuf[:, c0:c0 + cn],
                    rhs=W_sb[:],
                    start=True,
                    stop=True,
                )
                ck = ckpool.tile([P, P], bf16, tag="prodck")
                nc.scalar.activation(ck[:cn, :], ps[:cn, :], AF.Copy)
                nc.sync.dma_start(out=bounce[c0:c0 + cn, :], in_=ck[:cn, :])

        def aggregate(table, bias_sb, stag):
            """Gather + selection-matmul + relu epilogue into stag."""
            for b, (w0, bw) in enumerate(batches):
                gtiles = []
                for q in range(nreg):
                    if C[q] == 0:
                        gtiles.append(None)
                        continue
                    sub0 = w0 * SUBW + bw * offq[q]
                    ln = bw * C[q] * P
                    gb = gpool.tile([P, B * C[q] * D], bf16, tag=f"gbuf{q}")
                    nc.gpsimd.dma_gather(
                        gb[:, :ln // P * D].rearrange("p (c d) -> p c d", d=D),
                        table[regs[q][0]:regs[q][1], :],
                        idxm[:, sub0 * 8: sub0 * 8 + ln // 16],
                        ln,
                        ln,
                        D,
                        single_packet=False,
                    )
                    gtiles.append((gb, sub0))
                for wl in range(bw):
                    w = w0 + wl
                    n0 = w * WW
                    wn = min(WW, npc - n0)
                    ps = apspool.tile([P, WW], f32, space="PSUM", tag="aps")
                    k = 0
                    for q in range(nreg):
                        if C[q] == 0:
                            continue
                        gb, sub0 = gtiles[q]
                        for s in range(C[q]):
                            sub = sub0 + wl * C[q] + s
                            lsub = wl * C[q] + s
                            Sp = spool.tile([P, WW], bf16, tag="selm")
                            nc.vector.tensor_scalar(
                                Sp[:],
                                iot[:],
                                rm[:, sub:sub + 1],
                                num[:, sub:sub + 1],
                                ALU.is_equal,
                                ALU.mult,
                            )
                            nc.tensor.matmul(
                                out=ps[:],
                                lhsT=gb[:, lsub * D:(lsub + 1) * D],
                                rhs=Sp[:],
                                start=(k == 0),
                                stop=(k == SUBW - 1),
                            )
                            k += 1
                    nc.scalar.activation(
                        stag[:, n0:n0 + wn],
                        ps[:, :wn],
                        AF.Relu,
                        bias=bias_sb[:, 0:1],
                    )

        # ---------------- layer 1 ----------------
        production(xT, W1, bnc1)
        nc.gpsimd.collective_compute(
            "AllGather",
            ALU.bypass,
            replica_groups=rgroups,
            ins=[bnc1[0:npc, :].opt()],
            outs=[t1[0:NCORES * npc, :].opt()],
        )
        aggregate(t1, b1, stag1)

        # ---------------- layer 2 ----------------
        production(stag1, W2, bnc2)
        nc.gpsimd.collective_compute(
            "AllGather",
            ALU.bypass,
            replica_groups=rgroups,
            ins=[bnc2[0:npc, :].opt()],
            outs=[t2[0:NCORES * npc, :].opt()],
        )
        aggregate(t2, b2, stag2)

        # ---------------- residual + combine ----------------
        for rc in range(nrchunks):
            r0 = rc * RW
            cw = min(RW, npc - r0)
            ps = rpspool.tile([P, RW], f32, space="PSUM", tag="rps")
            nc.tensor.matmul(
                out=ps[:, :cw],
                lhsT=Wfc[:],
                rhs=xT[:, r0:r0 + cw],
                start=True,
                stop=True,
            )
            rb = ckpool.tile([P, RW], f32, tag="resck")
            nc.scalar.activation(
                rb[:, :cw], ps[:, :cw], AF.Identity, bias=bfc[:, 0:1]
            )
            ob = ckpool.tile([P, RW], f32, tag="outck")
            nc.vector.tensor_tensor(
                out=ob[:, :cw],
                in0=rb[:, :cw],
                in1=stag2[:, r0:r0 + cw],
                op=ALU.add,
            )
            nc.sync.dma_start(out=out_e[:, r0:r0 + cw], in_=ob[:, :cw])

    nc.compile()
    return nc


# --------------------------------------------------------------------------
# Entry point
# --------------------------------------------------------------------------

def _prep(x, edge_index, W1, b1, W2, b2, Wfc, bfc):
    N = x.shape[0]
    assert N % NCORES == 0
    npc = N // NCORES

    loop = np.arange(N, dtype=np.int64)
    src = np.concatenate([edge_index[0].astype(np.int64), loop])
    dst = np.concatenate([edge_index[1].astype(np.int64), loop])
    deg = np.bincount(dst, minlength=N).astype(np.float32)
    sigma = np.where(deg > 0, 1.0 / np.sqrt(deg), 0.0).astype(np.float32)
    nu = sigma[src] * sigma[dst]

    cores = [_core_edges(c, src, dst, nu, npc) for c in range(NCORES)]

    nreg = len(_regions(N))
    maxc = np.zeros(nreg, np.int64)
    for s_src, s_dst, s_nu in cores:
        cnt = _count_wr(s_src, s_dst, npc, N)
        maxc = np.maximum(maxc, cnt.max(axis=0))
    C = [int((m + P - 1) // P) for m in maxc]

    iota = np.tile(np.arange(WW, dtype=np.float32), (P, 1))
    W1b = np.asarray(W1, np.float32).astype(BF16)
    W2b = np.asarray(W2, np.float32).astype(BF16)
    Wfcb = np.asarray(Wfc, np.float32).astype(BF16)
    b1c = np.asarray(b1, np.float32).reshape(P, 1)
    b2c = np.asarray(b2, np.float32).reshape(P, 1)
    bfcc = np.asarray(bfc, np.float32).reshape(P, 1)

    in_maps = []
    for c in range(NCORES):
        s_src, s_dst, s_nu = cores[c]
        idx, r, nnu, counts = _finalize_core(s_src, s_dst, s_nu, npc, N, C)
        xTc = np.ascontiguousarray(x[c * npc:(c + 1) * npc].T.astype(BF16))
        in_maps.append({
            "xT": xTc,
            "W1": W1b, "W2": W2b, "Wfc": Wfcb,
            "b1": b1c, "b2": b2c, "bfc": bfcc,
            "iota": iota,
            "idx": idx, "r": r, "nu": nnu, "cnt": counts,
        })
    return in_maps, N, npc, C


def _ensure_ntff_hook():
    """The agent image's antenv lacks axon_hooks; shim it so trace=True
    works (falls back to hookless if the profiling lib is unavailable)."""
    try:
        import antenv.axon_hooks  # noqa: F401
        return
    except ImportError:
        pass
    try:
        import types

        import antenv

        mod = types.ModuleType("antenv.axon_hooks")
        _hook = [None]
        mod.set_axon_ntff_profile_hook = lambda h: _hook.__setitem__(0, h)
        mod.get_axon_ntff_profile_hook = lambda: _hook[0]
        sys.modules["antenv.axon_hooks"] = mod
        antenv.axon_hooks = mod
        try:
            from trn_agent_boot.trn_boot import _ntff_profile_via_ctypes

            mod.set_axon_ntff_profile_hook(
                _ntff_profile_via_ctypes("/opt/axon/libaxon_pjrt.so")
            )
        except Exception:
            pass
    except Exception:
        pass


def kernel(x, edge_index, W1, b1, W2, b2, Wfc, bfc):
    from concourse.bass_utils import run_bass_kernel_spmd

    x = np.asarray(x, np.float32)
    edge_index = np.asarray(edge_index)
    in_maps, N, npc, C = _prep(x, edge_index, W1, b1, W2, b2, Wfc, bfc)
    nc = _build_program(N, npc, C)

    trace = os.environ.get("GNN_TRACE", "0") == "1"
    if trace:
        _ensure_ntff_hook()
    res = run_bass_kernel_spmd(
        nc, in_maps, core_ids=list(range(NCORES)), trace=trace
    )
    _LAST_RESULTS["exec_time_ns"] = res.exec_time_ns
    _LAST_RESULTS["mean_exec_time_ns"] = res.mean_exec_time_ns
    _LAST_RESULTS["trace"] = res.instructions_and_trace

    out = np.concatenate(
        [res.results[c]["out"].T for c in range(NCORES)], axis=0
    )
    return np.ascontiguousarray(out.astype(np.float32))
